# revision 11
# baseline (speedup 1.0000x reference)
"""Trainium2 Bass kernel for BaselineProtonet (retrieval_knn).

logits[q, c] = -||query_q - proto_c||_2
  proto_c = mean of 64 support embeddings of class c
  embeddings_stacked: [64 classes * (64 support + 64 query), 1024] f32

Sharding (8 cores, 2D grid): 4 query blocks x 2 class blocks. Core
(i, j) owns query rows 1024*i..1024*(i+1) and classes 32*j..32*(j+1).
Per-core wire traffic: 2MB fp8 support + 2MB bf16 queries (queries must
be bf16: the ||q||^2 squares are elementwise on DVE, and fp8 runs at 1x
(1.2-3.5us per [128,1024] chunk, run-to-run unstable) while bf16 gets
the 2x packed mode (~0.85us). DMA streams at ~340GB/s in 512KB slices
(4KB/partition lines -- the SDMA engines are descriptor-latency bound,
so smaller lines halve throughput).

Host-side shard prep (layout/encoding only, no arithmetic): support
swizzled d-half-major with partition p owning class p//4 in every
256-row DoubleRow chunk; queries feature-major bf16.

Stream order and overlap (all compute tracks DMA completion sems, which
trail the last byte of a slice by ~2us):
  supH0 | qryA (d-half0) | supH1 | qryB (d-half1)
  - protos h0 (fp8 DoubleRow one-hot matmuls) right after warmup;
    evac (1/64) -> PE transpose -> ACT *-2 gives W16 for d-half0, so
    gram+||q||^2 matmuls for chunks 0-3 run mid-kernel
  - protos h1 + W chain B overlap the qryB stream; only chunk 4-7
    gram/ones matmuls, sqrt(+||p||^2), negate, store trail the last byte
  - dummy no-dep matmuls keep the PE HAM clock-gate warm across waits
"""

import numpy as np

C = 64          # classes
S = 64          # support per class (== queries per class)
D = 1024        # embedding dim
NCORES = 8
QB = 4          # query blocks
CB = 2          # class blocks
CL = C // CB    # 32 classes per core
QL = (C * S) // QB          # 1024 query rows per core
SJP = (CL * S) // 256       # 8 support chunk-pairs per core
DCH = D // 128              # 8 feature chunks

_CACHE = {}


def _emit(nc, tc, sup, qt, oh_in, out):
    """Emit the per-core tile program.

    sup:   [128, 2*SJP*2*512] fp8 DRAM (support, [h, jp, o, d512] cols,
                                        partition p owns class p//4)
    qt:    [128, DCH*QL] bf16 DRAM     (queries, feature-major)
    oh_in: [128, 2*CL] fp8 DRAM        (DoubleRow one-hot class mask)
    out:   [CL, QL] f32 DRAM           (negated distances, class-major)
    """
    from concourse import masks, mybir

    f32 = mybir.dt.float32
    bf16 = mybir.dt.bfloat16
    fp8 = mybir.dt.float8e4
    AF = mybir.ActivationFunctionType
    DR = mybir.MatmulPerfMode.DoubleRow

    with (
        tc.tile_pool(name="sb", bufs=1) as sb,
        tc.tile_pool(name="ps", bufs=1, space="PSUM") as ps,
    ):
        # warm the PE clock first-thing: HAM un-throttles only after
        # ~3.4us of sustained matmul activity
        wm_in = sb.tile([128, 512], bf16)
        nc.gpsimd.memset(wm_in[:], 0.0)
        wm_ps = ps.tile([128, 512], f32)
        for _ in range(8):
            nc.tensor.matmul(
                wm_ps[:], wm_in[:, 0:128], wm_in[:], start=True, stop=True
            )

        def dummy_mm(n=1):
            # no-dep matmuls: scheduled only when nothing real is ready,
            # they bridge DMA/evac waits so the HAM gate stays warm
            for _ in range(n):
                nc.tensor.matmul(
                    wm_ps[:], wm_in[:, 0:128], wm_in[:], start=True, stop=True
                )

        # ---------------- input DMA ring (one sync-queue FIFO) ----------
        oh = sb.tile([128, 2, CL], fp8)
        nc.scalar.dma_start(
            oh[:], oh_in[:, :].rearrange("p (o c) -> p o c", o=2)
        )
        q16 = sb.tile([128, DCH, QL], bf16)
        s8 = sb.tile([128, 2, SJP, 2, 512], fp8)

        def q_slice(g):  # 512KB: d-chunks 2g, 2g+1, all queries
            nc.sync.dma_start(
                q16[:, 2 * g : 2 * (g + 1)],
                qt[:, 2 * QL * g : 2 * QL * (g + 1)].rearrange(
                    "p (k q) -> p k q", k=2
                ),
            )

        def s_slice(h, jp0, njp):  # d-half h, chunk-pairs jp0..jp0+njp
            nc.sync.dma_start(
                s8[:, h, jp0 : jp0 + njp],
                sup[
                    :, 8192 * h + 1024 * jp0 : 8192 * h + 1024 * (jp0 + njp)
                ].rearrange("p (jp o d) -> p jp o d", jp=njp, o=2),
            )

        s_slice(0, 0, 4)    # supH0a: protos start right after warmup
        s_slice(0, 4, 4)    # supH0b
        q_slice(0)          # qryA1: d-chunks 0,1
        q_slice(1)          # qryA2: d-chunks 2,3
        s_slice(1, 0, 4)    # supH1a
        s_slice(1, 4, 4)    # supH1b
        q_slice(2)          # qryB1: d-chunks 4,5
        q_slice(3)          # qryB2: d-chunks 6,7

        # ---------------- constants -------------------------------------
        ident = sb.tile([128, 128], bf16)
        masks.make_identity(nc, ident[:])
        ones16 = sb.tile([128, CL], bf16)
        nc.gpsimd.memset(ones16[:], 1.0)

        # preload the sqrt ACT table off the critical path
        warm_sq = sb.tile([1, 1], f32)
        nc.gpsimd.memset(warm_sq[:], 1.0)
        nc.scalar.activation(warm_sq[:], warm_sq[:], AF.Sqrt)

        # ---------------- prototypes: d-half 0 --------------------------
        p_ps = ps.tile([CL, D], f32)  # [32, 1024] = 2 banks
        for jp in range(SJP):
            nc.tensor.matmul(
                p_ps[:, 0:512],
                oh[:],
                s8[:, 0, jp],
                start=(jp == 0),
                stop=(jp == SJP - 1),
                perf_mode=DR,
            )

        dummy_mm(4)

        # W chain: evac protos (1/64) to bf16, transpose per 128-d chunk,
        # scale by -2; quarter [32,256] evacs pipeline the chain
        psbA = sb.tile([CL, 512], bf16)
        psbB = sb.tile([CL, 512], bf16)
        pt_ps = ps.tile([128, DCH, CL], bf16)  # [128, 8*32] = 1 bank
        W16 = sb.tile([128, DCH, CL], bf16)

        def w_chain(h, psb):
            src_ps = p_ps[:, 512 * h : 512 * (h + 1)]
            for u in range(2):
                us = slice(256 * u, 256 * (u + 1))
                nc.scalar.mul(psb[:, us], src_ps[:, us], 1.0 / S)
                for kk in range(2):
                    k = 4 * h + 2 * u + kk
                    nc.tensor.transpose(
                        pt_ps[:, k],
                        psb[:, 256 * u + 128 * kk : 256 * u + 128 * (kk + 1)],
                        ident[0:CL, 0:CL],
                    )
                k0 = 4 * h + 2 * u
                nc.scalar.mul(
                    W16[:, k0 : k0 + 2].rearrange("p k c -> p (k c)"),
                    pt_ps[:, k0 : k0 + 2].rearrange("p k c -> p (k c)"),
                    -2.0,
                )

        w_chain(0, psbA)

        # ||q||^2 squares on DVE only (bf16 2x mode, ~0.85us per chunk)
        qsq = sb.tile([128, DCH, QL], bf16)

        def sq(k):
            nc.vector.tensor_mul(qsq[:, k], q16[:, k], q16[:, k])

        for k in range(4):
            sq(k)

        # gram + ||q||^2 matmuls for d-half0 run mid-kernel
        s_ps = ps.tile([CL, QL], f32)  # [32, 1024] = 2 banks

        def gram(k, start=False):
            for qh in range(2):
                qs = slice(512 * qh, 512 * (qh + 1))
                nc.tensor.matmul(
                    s_ps[:, qs], W16[:, k], q16[:, k, qs],
                    start=start, stop=False,
                )

        def ones(k, stop=False):
            for qh in range(2):
                qs = slice(512 * qh, 512 * (qh + 1))
                nc.tensor.matmul(
                    s_ps[:, qs], ones16[:], qsq[:, k, qs],
                    start=False, stop=stop,
                )

        gram(0, start=True)
        ones(0)
        gram(1)
        ones(1)
        gram(2)
        ones(2)
        gram(3)
        ones(3)

        # ||p||^2 half A on ACT (idle window), accumulate along d
        sq_dumpA = sb.tile([CL, 512], bf16)
        pnA = sb.tile([CL, 1], f32)
        nc.scalar.activation(sq_dumpA[:], psbA[:], AF.Square, accum_out=pnA[:])

        # ---------------- prototypes: d-half 1 --------------------------
        for jp in range(SJP):
            nc.tensor.matmul(
                p_ps[:, 512:1024],
                oh[:],
                s8[:, 1, jp],
                start=(jp == 0),
                stop=(jp == SJP - 1),
                perf_mode=DR,
            )
        dummy_mm(3)

        w_chain(1, psbB)

        for k in range(4, 8):
            sq(k)

        # tail matmuls for d-half1
        gram(4)
        ones(4)
        gram(5)
        ones(5)
        gram(6)
        ones(6)
        gram(7)
        ones(7, stop=True)

        # ||p||^2 half B + total
        sq_dumpB = sb.tile([CL, 512], bf16)
        pnB = sb.tile([CL, 1], f32)
        pn = sb.tile([CL, 1], f32)
        nc.scalar.activation(sq_dumpB[:], psbB[:], AF.Square, accum_out=pnB[:])
        nc.vector.tensor_add(pn[:], pnA[:], pnB[:])

        # ------- sqrt(+||p||^2), negate, store (2 q-halves pipelined) ---
        for qh in range(2):
            qs = slice(512 * qh, 512 * (qh + 1))
            lt = sb.tile([CL, 512], f32, name=f"lt{qh}")
            ltn = sb.tile([CL, 512], f32, name=f"ltn{qh}")
            nc.scalar.activation(lt[:], s_ps[:, qs], AF.Sqrt, bias=pn[:, 0:1])
            nc.vector.tensor_scalar_mul(ltn[:], lt[:], -1.0)
            nc.sync.dma_start(out[:, qs], ltn[:])


def _build():
    if "nc" in _CACHE:
        return _CACHE["nc"]
    from concourse import bacc, mybir, tile

    f32 = mybir.dt.float32
    bf16 = mybir.dt.bfloat16
    fp8 = mybir.dt.float8e4
    nc = bacc.Bacc(
        "TRN2",
        target_bir_lowering=False,
        debug=False,
        enable_asserts=False,
        num_devices=NCORES,
    )
    sup = nc.dram_tensor("sup", [128, 2 * SJP * 2 * 512], fp8, kind="ExternalInput").ap()
    qt = nc.dram_tensor("qt", [128, DCH * QL], bf16, kind="ExternalInput").ap()
    oh_in = nc.dram_tensor("oh", [128, 2 * CL], fp8, kind="ExternalInput").ap()
    out = nc.dram_tensor("out", [CL, QL], f32, kind="ExternalOutput").ap()
    with tile.TileContext(nc) as tc:
        _emit(nc, tc, sup, qt, oh_in, out)
    nc.compile()
    _CACHE["nc"] = nc
    return nc


def _shard(embeddings):
    import ml_dtypes

    emb = np.asarray(embeddings, dtype=np.float32).reshape(C, 2 * S, D)
    support = emb[:, :S, :]                       # [64, 64, 1024]
    queries = emb[:, S:, :].reshape(C * S, D)     # [4096, 1024]

    # one-hot: oh[p, o, c] = 1 iff c == p//4 (same for every chunk-pair)
    p = np.arange(128)[:, None, None]
    c = np.arange(CL)[None, None, :]
    oh = (c == p // 4) + np.zeros((1, 2, 1), dtype=bool)
    oh = np.ascontiguousarray(
        oh.astype(ml_dtypes.float8_e4m3).reshape(128, 2 * CL)
    )

    sups = []
    for j in range(CB):
        # [32, 64, 1024] -> [c, jp, o, m, h, 512] -> [c, m, h, jp, o, 512]
        sj = support[CL * j : CL * (j + 1)].reshape(CL, SJP, 2, 4, 2, 512)
        sj = sj.transpose(0, 3, 4, 1, 2, 5).reshape(128, 2 * SJP * 2 * 512)
        sups.append(np.ascontiguousarray(sj.astype(ml_dtypes.float8_e4m3)))
    qts = []
    for i in range(QB):
        # [1024q, 1024d] -> T -> [8k, 128p, 1024q] -> [p, k, q]
        qi = queries[QL * i : QL * (i + 1)].T.reshape(DCH, 128, QL)
        qi = qi.transpose(1, 0, 2).reshape(128, DCH * QL)
        qts.append(np.ascontiguousarray(qi.astype(ml_dtypes.bfloat16)))

    in_maps = []
    for k in range(NCORES):
        i, j = k // CB, k % CB
        in_maps.append({"sup": sups[j], "qt": qts[i], "oh": oh})
    return in_maps


def _gather(outs):
    """outs: list of 8 per-core [CL, QL] arrays -> full [C*S, C] f32."""
    logits = np.empty((C * S, C), dtype=np.float32)
    for k in range(NCORES):
        i, j = k // CB, k % CB
        logits[QL * i : QL * (i + 1), CL * j : CL * (j + 1)] = np.asarray(
            outs[k], dtype=np.float32
        ).T
    return logits


def kernel(embeddings_stacked, n_classes, n_support, **_unused):
    assert int(n_classes) == C and int(n_support) == S
    emb = np.asarray(embeddings_stacked)
    assert emb.shape == (C * 2 * S, D), emb.shape

    from concourse import bass_utils

    nc = _build()
    in_maps = _shard(emb)
    try:
        res = bass_utils.run_bass_kernel_spmd(
            nc, in_maps, core_ids=list(range(NCORES))
        )
    except Exception:
        # transient device/runtime hiccups have been observed; retry once
        res = bass_utils.run_bass_kernel_spmd(
            nc, in_maps, core_ids=list(range(NCORES))
        )
    return _gather([res.results[k]["out"] for k in range(NCORES)])


if __name__ == "__main__":
    rng = np.random.default_rng(0)
    emb = rng.standard_normal((C * 2 * S, D), dtype=np.float32)
    got = kernel(emb, C, S)
    print("kernel output", got.shape, got.dtype)


# revision 12
# speedup vs baseline: 1.0574x; 1.0574x over previous
"""Trainium2 Bass kernel for BaselineProtonet (retrieval_knn).

logits[q, c] = -||query_q - proto_c||_2
  proto_c = mean of 64 support embeddings of class c
  embeddings_stacked: [64 classes * (64 support + 64 query), 1024] f32

Sharding (8 cores, 2D grid): 4 query blocks x 2 class blocks. Core
(i, j) owns query rows 1024*i..1024*(i+1) and classes 32*j..32*(j+1).
Per-core wire traffic: 2MB fp8 support + 2MB bf16 queries (queries must
be bf16: the ||q||^2 squares are elementwise on DVE, and fp8 runs at 1x
(1.2-3.5us per [128,1024] chunk, run-to-run unstable) while bf16 gets
the 2x packed mode (~0.85us). DMA streams at ~340GB/s in 512KB slices
(4KB/partition lines -- the SDMA engines are descriptor-latency bound,
so smaller lines halve throughput).

Host-side shard prep (layout/encoding only, no arithmetic): support
swizzled d-half-major with partition p owning class p//4 in every
256-row DoubleRow chunk; queries feature-major bf16.

Stream order and overlap (all compute tracks DMA completion sems, which
trail the last byte of a slice by ~2us):
  supH0 | qryA (d-half0) | supH1 | qryB (d-half1)
  - protos h0 (fp8 DoubleRow one-hot matmuls) right after warmup;
    evac (1/64) -> PE transpose -> ACT *-2 gives W16 for d-half0, so
    gram+||q||^2 matmuls for chunks 0-3 run mid-kernel
  - protos h1 + W chain B overlap the qryB stream; only chunk 4-7
    gram/ones matmuls, sqrt(+||p||^2), negate, store trail the last byte
  - dummy no-dep matmuls keep the PE HAM clock-gate warm across waits
"""

import numpy as np

C = 64          # classes
S = 64          # support per class (== queries per class)
D = 1024        # embedding dim
NCORES = 8
QB = 4          # query blocks
CB = 2          # class blocks
CL = C // CB    # 32 classes per core
QL = (C * S) // QB          # 1024 query rows per core
SJP = (CL * S) // 256       # 8 support chunk-pairs per core
DCH = D // 128              # 8 feature chunks

_CACHE = {}


def _emit(nc, tc, sup, qt, oh_in, out):
    """Emit the per-core tile program.

    sup:   [128, 2*SJP*2*512] fp8 DRAM (support, [h, jp, o, d512] cols,
                                        partition p owns class p//4)
    qt:    [128, DCH*QL] bf16 DRAM     (queries, feature-major)
    oh_in: [128, 2*CL] fp8 DRAM        (DoubleRow one-hot class mask)
    out:   [CL, QL] bf16 DRAM          (negated distances, class-major)
    """
    from concourse import masks, mybir

    f32 = mybir.dt.float32
    bf16 = mybir.dt.bfloat16
    fp8 = mybir.dt.float8e4
    AF = mybir.ActivationFunctionType
    DR = mybir.MatmulPerfMode.DoubleRow

    with (
        tc.tile_pool(name="sb", bufs=1) as sb,
        tc.tile_pool(name="ps", bufs=1, space="PSUM") as ps,
    ):
        # warm the PE clock first-thing: HAM un-throttles only after
        # ~3.4us of sustained matmul activity
        wm_in = sb.tile([128, 512], bf16)
        nc.gpsimd.memset(wm_in[:], 0.0)
        wm_ps = ps.tile([128, 512], f32)
        for _ in range(8):
            nc.tensor.matmul(
                wm_ps[:], wm_in[:, 0:128], wm_in[:], start=True, stop=True
            )

        def dummy_mm(n=1):
            # no-dep matmuls: scheduled only when nothing real is ready,
            # they bridge DMA/evac waits so the HAM gate stays warm
            for _ in range(n):
                nc.tensor.matmul(
                    wm_ps[:], wm_in[:, 0:128], wm_in[:], start=True, stop=True
                )

        # ---------------- input DMA ring (one sync-queue FIFO) ----------
        oh = sb.tile([128, 2, CL], fp8)
        nc.scalar.dma_start(
            oh[:], oh_in[:, :].rearrange("p (o c) -> p o c", o=2)
        )
        q16 = sb.tile([128, DCH, QL], bf16)
        s8 = sb.tile([128, 2, SJP, 2, 512], fp8)

        def qa_slice(g):  # 512KB: d-chunks 2g, 2g+1, all queries
            nc.sync.dma_start(
                q16[:, 2 * g : 2 * (g + 1)],
                qt[:, 2 * QL * g : 2 * QL * (g + 1)].rearrange(
                    "p (k q) -> p k q", k=2
                ),
            )

        def qb_slice(col0, k0, nk, qh):  # d-chunks k0.., query-half qh
            nc.sync.dma_start(
                q16[:, k0 : k0 + nk, 512 * qh : 512 * (qh + 1)],
                qt[:, col0 : col0 + nk * 512].rearrange(
                    "p (k q) -> p k q", k=nk
                ),
            )

        def s_slice(h, jp0, njp):  # d-half h, chunk-pairs jp0..jp0+njp
            nc.sync.dma_start(
                s8[:, h, jp0 : jp0 + njp],
                sup[
                    :, 8192 * h + 1024 * jp0 : 8192 * h + 1024 * (jp0 + njp)
                ].rearrange("p (jp o d) -> p jp o d", jp=njp, o=2),
            )

        s_slice(0, 0, 4)    # supH0a: protos start right after warmup
        s_slice(0, 4, 4)    # supH0b
        qa_slice(0)         # qryA1: d-chunks 0,1
        qa_slice(1)         # qryA2: d-chunks 2,3
        s_slice(1, 0, 4)    # supH1a
        s_slice(1, 4, 4)    # supH1b
        # d-half1 queries split by query-half: bank qh0 gets all its
        # inputs one slice early, so its sqrt/negate/store pipeline
        # overlaps the final slices; the last 256KB slices keep the
        # trailing chain (2 squares + 2 matmuls + sqrt) minimal
        qb_slice(4096, 4, 4, 0)   # chunks 4-7, queries 0-511
        qb_slice(6144, 4, 2, 1)   # chunks 4,5, queries 512-1023
        qb_slice(7168, 6, 2, 1)   # chunks 6,7, queries 512-1023

        # ---------------- constants -------------------------------------
        ident = sb.tile([128, 128], bf16)
        masks.make_identity(nc, ident[:])
        ones16 = sb.tile([128, CL], bf16)
        nc.gpsimd.memset(ones16[:], 1.0)

        # preload the sqrt ACT table off the critical path
        warm_sq = sb.tile([1, 1], f32)
        nc.gpsimd.memset(warm_sq[:], 1.0)
        nc.scalar.activation(warm_sq[:], warm_sq[:], AF.Sqrt)

        # ---------------- prototypes: d-half 0 --------------------------
        p_ps = ps.tile([CL, D], f32)  # [32, 1024] = 2 banks
        for jp in range(SJP):
            nc.tensor.matmul(
                p_ps[:, 0:512],
                oh[:],
                s8[:, 0, jp],
                start=(jp == 0),
                stop=(jp == SJP - 1),
                perf_mode=DR,
            )

        dummy_mm(4)

        # W chain: evac protos (1/64) to bf16, transpose per 128-d chunk,
        # scale by -2; quarter [32,256] evacs pipeline the chain
        psbA = sb.tile([CL, 512], bf16)
        psbB = sb.tile([CL, 512], bf16)
        pt_ps = ps.tile([128, DCH, CL], bf16)  # [128, 8*32] = 1 bank
        W16 = sb.tile([128, DCH, CL], bf16)

        def w_chain(h, psb):
            src_ps = p_ps[:, 512 * h : 512 * (h + 1)]
            for u in range(2):
                us = slice(256 * u, 256 * (u + 1))
                nc.scalar.mul(psb[:, us], src_ps[:, us], 1.0 / S)
                for kk in range(2):
                    k = 4 * h + 2 * u + kk
                    nc.tensor.transpose(
                        pt_ps[:, k],
                        psb[:, 256 * u + 128 * kk : 256 * u + 128 * (kk + 1)],
                        ident[0:CL, 0:CL],
                    )
                k0 = 4 * h + 2 * u
                nc.scalar.mul(
                    W16[:, k0 : k0 + 2].rearrange("p k c -> p (k c)"),
                    pt_ps[:, k0 : k0 + 2].rearrange("p k c -> p (k c)"),
                    -2.0,
                )

        w_chain(0, psbA)

        # ||q||^2 squares on DVE only (bf16 2x mode, ~0.85us per chunk)
        qsq = sb.tile([128, DCH, QL], bf16)

        def sq(k, qh=None):
            qs = slice(0, QL) if qh is None else slice(512 * qh, 512 * (qh + 1))
            nc.vector.tensor_mul(qsq[:, k, qs], q16[:, k, qs], q16[:, k, qs])

        for k in range(4):
            sq(k)

        # gram + ||q||^2 matmuls for d-half0 run mid-kernel
        s_ps = ps.tile([CL, QL], f32)  # [32, 1024] = 2 banks

        def gram(k, start=False):
            for qh in range(2):
                qs = slice(512 * qh, 512 * (qh + 1))
                nc.tensor.matmul(
                    s_ps[:, qs], W16[:, k], q16[:, k, qs],
                    start=start, stop=False,
                )

        def ones(k, stop=False):
            for qh in range(2):
                qs = slice(512 * qh, 512 * (qh + 1))
                nc.tensor.matmul(
                    s_ps[:, qs], ones16[:], qsq[:, k, qs],
                    start=False, stop=stop,
                )

        gram(0, start=True)
        ones(0)
        gram(1)
        ones(1)
        gram(2)
        ones(2)
        gram(3)
        ones(3)

        # ||p||^2 half A on ACT (idle window), accumulate along d
        sq_dumpA = sb.tile([CL, 512], bf16)
        pnA = sb.tile([CL, 1], f32)
        nc.scalar.activation(sq_dumpA[:], psbA[:], AF.Square, accum_out=pnA[:])

        # ---------------- prototypes: d-half 1 --------------------------
        for jp in range(SJP):
            nc.tensor.matmul(
                p_ps[:, 512:1024],
                oh[:],
                s8[:, 1, jp],
                start=(jp == 0),
                stop=(jp == SJP - 1),
                perf_mode=DR,
            )
        dummy_mm(3)

        w_chain(1, psbB)

        for k in range(4, 8):
            sq(k, 0)
        sq(4, 1)
        sq(5, 1)

        # ||p||^2 half B on DVE (keeps the ACT queue clear for the
        # W-evacs and sqrts in the tail)
        pnsqB = sb.tile([CL, 512], bf16)
        pnB = sb.tile([CL, 1], f32)
        pn = sb.tile([CL, 1], f32)
        nc.vector.tensor_mul(pnsqB[:], psbB[:], psbB[:])
        nc.vector.tensor_reduce(
            pnB[:], pnsqB[:], axis=mybir.AxisListType.X, op=mybir.AluOpType.add
        )
        nc.vector.tensor_add(pn[:], pnA[:], pnB[:])

        sq(6, 1)
        sq(7, 1)

        # tail matmuls for d-half1
        gram(4)
        ones(4)
        gram(5)
        ones(5)
        gram(6)
        ones(6)
        gram(7)
        ones(7, stop=True)

        # ------- sqrt(+||p||^2), negate, store (2 q-halves pipelined) ---
        for qh in range(2):
            qs = slice(512 * qh, 512 * (qh + 1))
            lt = sb.tile([CL, 512], f32, name=f"lt{qh}")
            ltn = sb.tile([CL, 512], bf16, name=f"ltn{qh}")
            nc.scalar.activation(lt[:], s_ps[:, qs], AF.Sqrt, bias=pn[:, 0:1])
            nc.vector.tensor_scalar_mul(ltn[:], lt[:], -1.0)
            nc.sync.dma_start(out[:, qs], ltn[:])


def _build():
    if "nc" in _CACHE:
        return _CACHE["nc"]
    from concourse import bacc, mybir, tile

    f32 = mybir.dt.float32
    bf16 = mybir.dt.bfloat16
    fp8 = mybir.dt.float8e4
    nc = bacc.Bacc(
        "TRN2",
        target_bir_lowering=False,
        debug=False,
        enable_asserts=False,
        num_devices=NCORES,
    )
    sup = nc.dram_tensor("sup", [128, 2 * SJP * 2 * 512], fp8, kind="ExternalInput").ap()
    qt = nc.dram_tensor("qt", [128, DCH * QL], bf16, kind="ExternalInput").ap()
    oh_in = nc.dram_tensor("oh", [128, 2 * CL], fp8, kind="ExternalInput").ap()
    out = nc.dram_tensor("out", [CL, QL], bf16, kind="ExternalOutput").ap()
    with tile.TileContext(nc) as tc:
        _emit(nc, tc, sup, qt, oh_in, out)
    nc.compile()
    _CACHE["nc"] = nc
    return nc


def _shard(embeddings):
    import ml_dtypes

    emb = np.asarray(embeddings, dtype=np.float32).reshape(C, 2 * S, D)
    support = emb[:, :S, :]                       # [64, 64, 1024]
    queries = emb[:, S:, :].reshape(C * S, D)     # [4096, 1024]

    # one-hot: oh[p, o, c] = 1 iff c == p//4 (same for every chunk-pair)
    p = np.arange(128)[:, None, None]
    c = np.arange(CL)[None, None, :]
    oh = (c == p // 4) + np.zeros((1, 2, 1), dtype=bool)
    oh = np.ascontiguousarray(
        oh.astype(ml_dtypes.float8_e4m3).reshape(128, 2 * CL)
    )

    sups = []
    for j in range(CB):
        # [32, 64, 1024] -> [c, jp, o, m, h, 512] -> [c, m, h, jp, o, 512]
        sj = support[CL * j : CL * (j + 1)].reshape(CL, SJP, 2, 4, 2, 512)
        sj = sj.transpose(0, 3, 4, 1, 2, 5).reshape(128, 2 * SJP * 2 * 512)
        sups.append(np.ascontiguousarray(sj.astype(ml_dtypes.float8_e4m3)))
    qts = []
    for i in range(QB):
        # [1024q, 1024d] -> T -> [8k, 128p, 1024q] -> [p, k, q]
        qi = queries[QL * i : QL * (i + 1)].T.reshape(DCH, 128, QL)
        qi = qi.transpose(1, 0, 2)               # [p, k, q]
        qa = qi[:, 0:4].reshape(128, 4 * QL)     # d-half0: k-major
        qb = qi[:, 4:8].reshape(128, 4, 2, 512)  # d-half1: [k, qh, q]
        qb = qb.transpose(0, 2, 1, 3).reshape(128, 4 * QL)  # [qh, k, q]
        qf = np.concatenate([qa, qb], axis=1)
        qts.append(np.ascontiguousarray(qf.astype(ml_dtypes.bfloat16)))

    in_maps = []
    for k in range(NCORES):
        i, j = k // CB, k % CB
        in_maps.append({"sup": sups[j], "qt": qts[i], "oh": oh})
    return in_maps


def _gather(outs):
    """outs: list of 8 per-core [CL, QL] arrays -> full [C*S, C] f32."""
    logits = np.empty((C * S, C), dtype=np.float32)
    for k in range(NCORES):
        i, j = k // CB, k % CB
        logits[QL * i : QL * (i + 1), CL * j : CL * (j + 1)] = np.asarray(
            outs[k], dtype=np.float32
        ).T
    return logits


def kernel(embeddings_stacked, n_classes, n_support, **_unused):
    assert int(n_classes) == C and int(n_support) == S
    emb = np.asarray(embeddings_stacked)
    assert emb.shape == (C * 2 * S, D), emb.shape

    from concourse import bass_utils

    nc = _build()
    in_maps = _shard(emb)
    try:
        res = bass_utils.run_bass_kernel_spmd(
            nc, in_maps, core_ids=list(range(NCORES))
        )
    except Exception:
        # transient device/runtime hiccups have been observed; retry once
        res = bass_utils.run_bass_kernel_spmd(
            nc, in_maps, core_ids=list(range(NCORES))
        )
    return _gather([res.results[k]["out"] for k in range(NCORES)])


if __name__ == "__main__":
    rng = np.random.default_rng(0)
    emb = rng.standard_normal((C * 2 * S, D), dtype=np.float32)
    got = kernel(emb, C, S)
    print("kernel output", got.shape, got.dtype)


# revision 13
# speedup vs baseline: 1.0706x; 1.0125x over previous
"""Trainium2 Bass kernel for BaselineProtonet (retrieval_knn).

logits[q, c] = -||query_q - proto_c||_2
  proto_c = mean of 64 support embeddings of class c
  embeddings_stacked: [64 classes * (64 support + 64 query), 1024] f32

Sharding (8 cores, 2D grid): 4 query blocks x 2 class blocks. Core
(i, j) owns query rows 1024*i..1024*(i+1) and classes 32*j..32*(j+1).
Per-core wire traffic: 2MB fp8 support + 2MB bf16 queries (queries must
be bf16: the ||q||^2 squares are elementwise on DVE, and fp8 runs at 1x
(1.2-3.5us per [128,1024] chunk, run-to-run unstable) while bf16 gets
the 2x packed mode (~0.85us). DMA streams at ~340GB/s in 512KB slices
(4KB/partition lines -- the SDMA engines are descriptor-latency bound,
so smaller lines halve throughput).

Host-side shard prep (layout/encoding only, no arithmetic): support
swizzled d-half-major with partition p owning class p//4 in every
256-row DoubleRow chunk; queries feature-major bf16.

Stream order and overlap (all compute tracks DMA completion sems, which
trail the last byte of a slice by ~2us):
  supH0 | qryA (d-half0) | supH1 | qryB (d-half1)
  - protos h0 (fp8 DoubleRow one-hot matmuls) right after warmup;
    evac (1/64) -> PE transpose -> ACT *-2 gives W16 for d-half0, so
    gram+||q||^2 matmuls for chunks 0-3 run mid-kernel
  - protos h1 + W chain B overlap the qryB stream; only chunk 4-7
    gram/ones matmuls, sqrt(+||p||^2), negate, store trail the last byte
  - dummy no-dep matmuls keep the PE HAM clock-gate warm across waits
"""

import numpy as np

C = 64          # classes
S = 64          # support per class (== queries per class)
D = 1024        # embedding dim
NCORES = 8
QB = 4          # query blocks
CB = 2          # class blocks
CL = C // CB    # 32 classes per core
QL = (C * S) // QB          # 1024 query rows per core
SJP = (CL * S) // 256       # 8 support chunk-pairs per core
DCH = D // 128              # 8 feature chunks

_CACHE = {}


def _emit(nc, tc, sup, qt, oh_in, out):
    """Emit the per-core tile program.

    sup:   [128, 2*SJP*2*512] fp8 DRAM (support, [h, jp, o, d512] cols,
                                        partition p owns class p//4)
    qt:    [128, DCH*QL] bf16 DRAM     (queries, feature-major)
    oh_in: [128, 2*CL] fp8 DRAM        (DoubleRow one-hot class mask)
    out:   [CL, QL] bf16 DRAM          (negated distances, class-major)
    """
    from concourse import masks, mybir

    f32 = mybir.dt.float32
    bf16 = mybir.dt.bfloat16
    fp8 = mybir.dt.float8e4
    AF = mybir.ActivationFunctionType
    DR = mybir.MatmulPerfMode.DoubleRow

    with (
        tc.tile_pool(name="sb", bufs=1) as sb,
        tc.tile_pool(name="ps", bufs=1, space="PSUM") as ps,
    ):
        # warm the PE clock first-thing: HAM un-throttles only after
        # ~3.4us of sustained matmul activity
        wm_in = sb.tile([128, 512], bf16)
        nc.gpsimd.memset(wm_in[:], 0.0)
        wm_ps = ps.tile([128, 512], f32)
        for _ in range(8):
            nc.tensor.matmul(
                wm_ps[:], wm_in[:, 0:128], wm_in[:], start=True, stop=True
            )

        def dummy_mm(n=1):
            # no-dep matmuls: scheduled only when nothing real is ready,
            # they bridge DMA/evac waits so the HAM gate stays warm
            for _ in range(n):
                nc.tensor.matmul(
                    wm_ps[:], wm_in[:, 0:128], wm_in[:], start=True, stop=True
                )

        # ---------------- input DMA ring (one sync-queue FIFO) ----------
        oh = sb.tile([128, 2, CL], fp8)
        nc.scalar.dma_start(
            oh[:], oh_in[:, :].rearrange("p (o c) -> p o c", o=2)
        )
        # [p, query-half, d-chunk, 512q]: every DMA slice below lands as
        # one contiguous per-partition run (strided SBUF writes quarter
        # the DMA rate)
        q16 = sb.tile([128, 2, DCH, 512], bf16)
        s8 = sb.tile([128, 2, SJP, 2, 512], fp8)

        def q_slice(qh, k0, nk):  # d-chunks k0..k0+nk, query-half qh
            col0 = 4096 * qh + 512 * k0
            nc.sync.dma_start(
                q16[:, qh, k0 : k0 + nk],
                qt[:, col0 : col0 + nk * 512].rearrange(
                    "p (k q) -> p k q", k=nk
                ),
            )

        def s_slice(h, jp0, njp):  # d-half h, chunk-pairs jp0..jp0+njp
            nc.sync.dma_start(
                s8[:, h, jp0 : jp0 + njp],
                sup[
                    :, 8192 * h + 1024 * jp0 : 8192 * h + 1024 * (jp0 + njp)
                ].rearrange("p (jp o d) -> p jp o d", jp=njp, o=2),
            )

        s_slice(0, 0, 4)    # supH0a: protos start right after warmup
        s_slice(0, 4, 4)    # supH0b
        q_slice(0, 0, 4)    # d-chunks 0-3, queries 0-511
        q_slice(1, 0, 4)    # d-chunks 0-3, queries 512-1023
        s_slice(1, 0, 4)    # supH1a
        s_slice(1, 4, 4)    # supH1b
        # d-half1 queries split by query-half: bank qh0 gets all its
        # inputs one slice early, so its sqrt/negate/store pipeline
        # overlaps the final slices; the last 256KB slices keep the
        # trailing chain (2 squares + 2 matmuls + sqrt) minimal
        q_slice(0, 4, 4)    # d-chunks 4-7, queries 0-511
        q_slice(1, 4, 2)    # d-chunks 4,5, queries 512-1023
        q_slice(1, 6, 2)    # d-chunks 6,7, queries 512-1023

        # ---------------- constants -------------------------------------
        ident = sb.tile([128, 128], bf16)
        masks.make_identity(nc, ident[:])
        ones16 = sb.tile([128, CL], bf16)
        nc.gpsimd.memset(ones16[:], 1.0)

        # preload the sqrt ACT table off the critical path
        warm_sq = sb.tile([1, 1], f32)
        nc.gpsimd.memset(warm_sq[:], 1.0)
        nc.scalar.activation(warm_sq[:], warm_sq[:], AF.Sqrt)

        # ---------------- prototypes: d-half 0 --------------------------
        p_ps = ps.tile([CL, D], f32)  # [32, 1024] = 2 banks
        for jp in range(SJP):
            nc.tensor.matmul(
                p_ps[:, 0:512],
                oh[:],
                s8[:, 0, jp],
                start=(jp == 0),
                stop=(jp == SJP - 1),
                perf_mode=DR,
            )

        dummy_mm(4)

        # W chain: evac protos (1/64) to bf16, transpose per 128-d chunk,
        # scale by -2; quarter [32,256] evacs pipeline the chain
        psbA = sb.tile([CL, 512], bf16)
        psbB = sb.tile([CL, 512], bf16)
        pt_ps = ps.tile([128, DCH, CL], bf16)  # [128, 8*32] = 1 bank
        W16 = sb.tile([128, DCH, CL], bf16)

        def w_chain(h, psb):
            src_ps = p_ps[:, 512 * h : 512 * (h + 1)]
            for u in range(2):
                us = slice(256 * u, 256 * (u + 1))
                nc.scalar.mul(psb[:, us], src_ps[:, us], 1.0 / S)
                for kk in range(2):
                    k = 4 * h + 2 * u + kk
                    nc.tensor.transpose(
                        pt_ps[:, k],
                        psb[:, 256 * u + 128 * kk : 256 * u + 128 * (kk + 1)],
                        ident[0:CL, 0:CL],
                    )
                k0 = 4 * h + 2 * u
                nc.scalar.mul(
                    W16[:, k0 : k0 + 2].rearrange("p k c -> p (k c)"),
                    pt_ps[:, k0 : k0 + 2].rearrange("p k c -> p (k c)"),
                    -2.0,
                )

        w_chain(0, psbA)

        # ||q||^2 squares on DVE only (bf16 2x mode, ~0.85us per chunk)
        qsq = sb.tile([128, 2, DCH, 512], bf16)

        def sq(k, qh):
            nc.vector.tensor_mul(qsq[:, qh, k], q16[:, qh, k], q16[:, qh, k])

        for k in range(4):
            sq(k, 0)
            sq(k, 1)

        # gram + ||q||^2 matmuls for d-half0 run mid-kernel
        s_ps = ps.tile([CL, QL], f32)  # [32, 1024] = 2 banks

        def gram(k, start=False):
            for qh in range(2):
                qs = slice(512 * qh, 512 * (qh + 1))
                nc.tensor.matmul(
                    s_ps[:, qs], W16[:, k], q16[:, qh, k],
                    start=start, stop=False,
                )

        def ones(k, stop=False):
            for qh in range(2):
                qs = slice(512 * qh, 512 * (qh + 1))
                nc.tensor.matmul(
                    s_ps[:, qs], ones16[:], qsq[:, qh, k],
                    start=False, stop=stop,
                )

        gram(0, start=True)
        ones(0)
        gram(1)
        ones(1)
        gram(2)
        ones(2)
        gram(3)
        ones(3)

        # ||p||^2 half A on ACT (idle window), accumulate along d
        sq_dumpA = sb.tile([CL, 512], bf16)
        pnA = sb.tile([CL, 1], f32)
        nc.scalar.activation(sq_dumpA[:], psbA[:], AF.Square, accum_out=pnA[:])

        # ---------------- prototypes: d-half 1 --------------------------
        for jp in range(SJP):
            nc.tensor.matmul(
                p_ps[:, 512:1024],
                oh[:],
                s8[:, 1, jp],
                start=(jp == 0),
                stop=(jp == SJP - 1),
                perf_mode=DR,
            )
        dummy_mm(3)

        w_chain(1, psbB)

        for k in range(4, 8):
            sq(k, 0)
        sq(4, 1)
        sq(5, 1)

        # ||p||^2 half B on DVE (keeps the ACT queue clear for the
        # W-evacs and sqrts in the tail)
        pnsqB = sb.tile([CL, 512], bf16)
        pnB = sb.tile([CL, 1], f32)
        pn = sb.tile([CL, 1], f32)
        nc.vector.tensor_mul(pnsqB[:], psbB[:], psbB[:])
        nc.vector.tensor_reduce(
            pnB[:], pnsqB[:], axis=mybir.AxisListType.X, op=mybir.AluOpType.add
        )
        nc.vector.tensor_add(pn[:], pnA[:], pnB[:])

        sq(6, 1)
        sq(7, 1)

        # tail matmuls for d-half1
        gram(4)
        ones(4)
        gram(5)
        ones(5)
        gram(6)
        ones(6)
        gram(7)
        ones(7, stop=True)

        # ------- sqrt(+||p||^2), negate, store (2 q-halves pipelined) ---
        for qh in range(2):
            qs = slice(512 * qh, 512 * (qh + 1))
            lt = sb.tile([CL, 512], f32, name=f"lt{qh}")
            ltn = sb.tile([CL, 512], bf16, name=f"ltn{qh}")
            nc.scalar.activation(lt[:], s_ps[:, qs], AF.Sqrt, bias=pn[:, 0:1])
            nc.vector.tensor_scalar_mul(ltn[:], lt[:], -1.0)
            nc.sync.dma_start(out[:, qs], ltn[:])


def _build():
    if "nc" in _CACHE:
        return _CACHE["nc"]
    from concourse import bacc, mybir, tile

    f32 = mybir.dt.float32
    bf16 = mybir.dt.bfloat16
    fp8 = mybir.dt.float8e4
    nc = bacc.Bacc(
        "TRN2",
        target_bir_lowering=False,
        debug=False,
        enable_asserts=False,
        num_devices=NCORES,
    )
    sup = nc.dram_tensor("sup", [128, 2 * SJP * 2 * 512], fp8, kind="ExternalInput").ap()
    qt = nc.dram_tensor("qt", [128, DCH * QL], bf16, kind="ExternalInput").ap()
    oh_in = nc.dram_tensor("oh", [128, 2 * CL], fp8, kind="ExternalInput").ap()
    out = nc.dram_tensor("out", [CL, QL], bf16, kind="ExternalOutput").ap()
    with tile.TileContext(nc) as tc:
        _emit(nc, tc, sup, qt, oh_in, out)
    nc.compile()
    _CACHE["nc"] = nc
    return nc


def _shard(embeddings):
    import ml_dtypes

    emb = np.asarray(embeddings, dtype=np.float32).reshape(C, 2 * S, D)
    support = emb[:, :S, :]                       # [64, 64, 1024]
    queries = emb[:, S:, :].reshape(C * S, D)     # [4096, 1024]

    # one-hot: oh[p, o, c] = 1 iff c == p//4 (same for every chunk-pair)
    p = np.arange(128)[:, None, None]
    c = np.arange(CL)[None, None, :]
    oh = (c == p // 4) + np.zeros((1, 2, 1), dtype=bool)
    oh = np.ascontiguousarray(
        oh.astype(ml_dtypes.float8_e4m3).reshape(128, 2 * CL)
    )

    sups = []
    for j in range(CB):
        # [32, 64, 1024] -> [c, jp, o, m, h, 512] -> [c, m, h, jp, o, 512]
        sj = support[CL * j : CL * (j + 1)].reshape(CL, SJP, 2, 4, 2, 512)
        sj = sj.transpose(0, 3, 4, 1, 2, 5).reshape(128, 2 * SJP * 2 * 512)
        sups.append(np.ascontiguousarray(sj.astype(ml_dtypes.float8_e4m3)))
    qts = []
    for i in range(QB):
        # [1024q, 1024d] -> T -> [8k, 128p, 1024q] -> [p, k, q]
        qi = queries[QL * i : QL * (i + 1)].T.reshape(DCH, 128, 2, 512)
        qi = qi.transpose(1, 2, 0, 3).reshape(128, DCH * QL)  # [p, qh, k, q]
        qts.append(np.ascontiguousarray(qi.astype(ml_dtypes.bfloat16)))

    in_maps = []
    for k in range(NCORES):
        i, j = k // CB, k % CB
        in_maps.append({"sup": sups[j], "qt": qts[i], "oh": oh})
    return in_maps


def _gather(outs):
    """outs: list of 8 per-core [CL, QL] arrays -> full [C*S, C] f32."""
    logits = np.empty((C * S, C), dtype=np.float32)
    for k in range(NCORES):
        i, j = k // CB, k % CB
        logits[QL * i : QL * (i + 1), CL * j : CL * (j + 1)] = np.asarray(
            outs[k], dtype=np.float32
        ).T
    return logits


def kernel(embeddings_stacked, n_classes, n_support, **_unused):
    assert int(n_classes) == C and int(n_support) == S
    emb = np.asarray(embeddings_stacked)
    assert emb.shape == (C * 2 * S, D), emb.shape

    from concourse import bass_utils

    nc = _build()
    in_maps = _shard(emb)
    try:
        res = bass_utils.run_bass_kernel_spmd(
            nc, in_maps, core_ids=list(range(NCORES))
        )
    except Exception:
        # transient device/runtime hiccups have been observed; retry once
        res = bass_utils.run_bass_kernel_spmd(
            nc, in_maps, core_ids=list(range(NCORES))
        )
    return _gather([res.results[k]["out"] for k in range(NCORES)])


if __name__ == "__main__":
    rng = np.random.default_rng(0)
    emb = rng.standard_normal((C * 2 * S, D), dtype=np.float32)
    got = kernel(emb, C, S)
    print("kernel output", got.shape, got.dtype)


# revision 19
# speedup vs baseline: 1.0822x; 1.0108x over previous
"""Trainium2 Bass kernel for BaselineProtonet (retrieval_knn).

logits[q, c] = -||query_q - proto_c||_2
  proto_c = mean of 64 support embeddings of class c
  embeddings_stacked: [64 classes * (64 support + 64 query), 1024] f32

Sharding (8 cores, 2D grid): 4 query blocks x 2 class blocks. Core
(i, j) owns query rows 1024*i..1024*(i+1) and classes 32*j..32*(j+1).
Per-core wire traffic: 2MB fp8 support + 2MB bf16 queries (queries must
be bf16: the ||q||^2 squares are elementwise on DVE, and fp8 runs at 1x
(1.2-3.5us per [128,1024] chunk, run-to-run unstable) while bf16 gets
the 2x packed mode (~0.85us). DMA streams at ~340GB/s in 512KB slices
(4KB/partition lines -- the SDMA engines are descriptor-latency bound,
so smaller lines halve throughput).

Host-side shard prep (layout/encoding only, no arithmetic): support
swizzled d-half-major with partition p owning class p//4 in every
256-row DoubleRow chunk; queries feature-major bf16.

Stream order and overlap (all compute tracks DMA completion sems, which
trail the last byte of a slice by ~2us):
  supH0 | qryA (d-half0) | supH1 | qryB (d-half1)
  - protos h0 (fp8 DoubleRow one-hot matmuls) right after warmup;
    evac (1/64) -> PE transpose -> ACT *-2 gives W16 for d-half0, so
    gram+||q||^2 matmuls for chunks 0-3 run mid-kernel
  - protos h1 + W chain B overlap the qryB stream; only chunk 4-7
    gram/ones matmuls, sqrt(+||p||^2), negate, store trail the last byte
  - dummy no-dep matmuls keep the PE HAM clock-gate warm across waits
"""

import numpy as np

C = 64          # classes
S = 64          # support per class (== queries per class)
D = 1024        # embedding dim
NCORES = 8
QB = 4          # query blocks
CB = 2          # class blocks
CL = C // CB    # 32 classes per core
QL = (C * S) // QB          # 1024 query rows per core
SJP = (CL * S) // 256       # 8 support chunk-pairs per core
DCH = D // 128              # 8 feature chunks

_CACHE = {}


def _emit(nc, tc, sup, qt, oh_in, out):
    """Emit the per-core tile program.

    sup:   [128, 2*SJP*2*512] fp8 DRAM (support, [h, jp, o, d512] cols,
                                        partition p owns class p//4)
    qt:    [128, DCH*QL] bf16 DRAM     (queries, feature-major)
    oh_in: [128, 2*CL] fp8 DRAM        (DoubleRow one-hot class mask)
    out:   [CL, QL] bf16 DRAM          (negated distances, class-major)
    """
    from concourse import masks, mybir

    f32 = mybir.dt.float32
    bf16 = mybir.dt.bfloat16
    fp8 = mybir.dt.float8e4
    AF = mybir.ActivationFunctionType
    DR = mybir.MatmulPerfMode.DoubleRow

    with (
        tc.tile_pool(name="sb", bufs=1) as sb,
        tc.tile_pool(name="ps", bufs=1, space="PSUM") as ps,
    ):
        # warm the PE clock first-thing: HAM un-throttles only after
        # ~3.4us of sustained matmul activity
        wm_in = sb.tile([128, 512], bf16)
        nc.gpsimd.memset(wm_in[:], 0.0)
        wm_ps = ps.tile([128, 512], f32)
        for _ in range(8):
            nc.tensor.matmul(
                wm_ps[:], wm_in[:, 0:128], wm_in[:], start=True, stop=True
            )

        def dummy_mm(n=1):
            # no-dep matmuls: scheduled only when nothing real is ready,
            # they bridge DMA/evac waits so the HAM gate stays warm
            for _ in range(n):
                nc.tensor.matmul(
                    wm_ps[:], wm_in[:, 0:128], wm_in[:], start=True, stop=True
                )

        # ---------------- input DMA ring (one sync-queue FIFO) ----------
        oh = sb.tile([128, 2, CL], fp8)
        nc.scalar.dma_start(
            oh[:], oh_in[:, :].rearrange("p (o c) -> p o c", o=2)
        )
        # [p, query-half, d-chunk, 512q]: every DMA slice below lands as
        # one contiguous per-partition run (strided SBUF writes quarter
        # the DMA rate)
        q16 = sb.tile([128, 2, DCH, 512], bf16)
        # support regions: d-half0 (cols 0-511), then two d-quarters
        # (512-767, 768-1023) so the final W-chain covers only 256 d-cols
        s8 = sb.tile([128, SJP, 2, 512], fp8)
        s8q = sb.tile([128, 2, SJP, 2, 256], fp8)

        def q_slice(qh, k0, nk):  # d-chunks k0..k0+nk, query-half qh
            col0 = 4096 * qh + 512 * k0
            nc.sync.dma_start(
                q16[:, qh, k0 : k0 + nk],
                qt[:, col0 : col0 + nk * 512].rearrange(
                    "p (k q) -> p k q", k=nk
                ),
            )

        def s_slice(jp0, njp):  # d-half0, chunk-pairs jp0..jp0+njp
            nc.sync.dma_start(
                s8[:, jp0 : jp0 + njp],
                sup[
                    :, 1024 * jp0 : 1024 * (jp0 + njp)
                ].rearrange("p (jp o d) -> p jp o d", jp=njp, o=2),
            )

        def s_qslice(r):  # d-quarter r (all 8 chunk-pairs), 512KB
            nc.sync.dma_start(
                s8q[:, r],
                sup[:, 8192 + 4096 * r : 8192 + 4096 * (r + 1)].rearrange(
                    "p (jp o d) -> p jp o d", jp=SJP, o=2
                ),
            )

        s_slice(0, 4)       # supH0a: protos start right after warmup
        s_slice(4, 4)       # supH0b
        q_slice(0, 0, 4)    # d-chunks 0-3, queries 0-511
        q_slice(1, 0, 4)    # d-chunks 0-3, queries 512-1023
        s_qslice(0)         # supQ1: d-cols 512-767, all chunk-pairs
        s_qslice(1)         # supQ2: d-cols 768-1023, all chunk-pairs
        # d-half1 queries split by query-half: bank qh0 gets all its
        # inputs one slice early, so its sqrt/negate/store pipeline
        # overlaps the final slices; the last 256KB slices keep the
        # trailing chain (2 squares + 2 matmuls + sqrt) minimal
        q_slice(0, 4, 4)    # d-chunks 4-7, queries 0-511
        q_slice(1, 4, 2)    # d-chunks 4,5, queries 512-1023
        q_slice(1, 6, 2)    # d-chunks 6,7, queries 512-1023

        # ---------------- constants -------------------------------------
        ident = sb.tile([128, 128], bf16)
        masks.make_identity(nc, ident[:])
        ones16 = sb.tile([128, CL], bf16)
        nc.gpsimd.memset(ones16[:], 1.0)

        # preload the sqrt ACT table off the critical path
        warm_sq = sb.tile([1, 1], f32)
        nc.gpsimd.memset(warm_sq[:], 1.0)
        nc.scalar.activation(warm_sq[:], warm_sq[:], AF.Sqrt)

        # ---------------- prototypes: d-half 0 --------------------------
        p_ps = ps.tile([CL, 512], f32)   # d-half0, 1 bank
        p_q1 = ps.tile([CL, 256], f32)   # d-cols 512-767
        p_q2 = ps.tile([CL, 256], f32)   # d-cols 768-1023
        for jp in range(SJP):
            nc.tensor.matmul(
                p_ps[:],
                oh[:],
                s8[:, jp],
                start=(jp == 0),
                stop=(jp == SJP - 1),
                perf_mode=DR,
            )

        dummy_mm(4)

        # W chain: evac protos (1/64) to bf16, transpose per 128-d chunk,
        # scale by -2; quarter [32,256] evacs pipeline the chain
        psbA = sb.tile([CL, 512], bf16)
        psbB = sb.tile([CL, 512], bf16)
        pt_ps = ps.tile([128, DCH, CL], bf16)  # [128, 8*32] = 1 bank
        W16 = sb.tile([128, DCH, CL], bf16)

        def w_quarter(src_ps, psb, us, k0):
            # evac [32,256] -> 2 transposes -> W scale for chunks k0,k0+1
            nc.scalar.mul(psb[:, us], src_ps, 1.0 / S)
            for kk in range(2):
                nc.tensor.transpose(
                    pt_ps[:, k0 + kk],
                    psb[:, us.start + 128 * kk : us.start + 128 * (kk + 1)],
                    ident[0:CL, 0:CL],
                )
            nc.scalar.mul(
                W16[:, k0 : k0 + 2].rearrange("p k c -> p (k c)"),
                pt_ps[:, k0 : k0 + 2].rearrange("p k c -> p (k c)"),
                -2.0,
            )

        w_quarter(p_ps[:, 0:256], psbA, slice(0, 256), 0)
        w_quarter(p_ps[:, 256:512], psbA, slice(256, 512), 2)

        # ||q||^2 squares on DVE only (bf16 2x mode, ~0.85us per chunk)
        qsq = sb.tile([128, 2, DCH, 512], bf16)

        def sq(k, qh):
            nc.vector.tensor_mul(qsq[:, qh, k], q16[:, qh, k], q16[:, qh, k])

        for k in range(4):
            sq(k, 0)
            sq(k, 1)

        # gram + ||q||^2 matmuls for d-half0 run mid-kernel
        s_ps = ps.tile([CL, QL], f32)  # [32, 1024] = 2 banks

        def gram(k, start=False):
            for qh in range(2):
                qs = slice(512 * qh, 512 * (qh + 1))
                nc.tensor.matmul(
                    s_ps[:, qs], W16[:, k], q16[:, qh, k],
                    start=start, stop=False,
                )

        def ones(k, stop=False):
            for qh in range(2):
                qs = slice(512 * qh, 512 * (qh + 1))
                nc.tensor.matmul(
                    s_ps[:, qs], ones16[:], qsq[:, qh, k],
                    start=False, stop=stop,
                )

        gram(0, start=True)
        ones(0)
        gram(1)
        ones(1)
        gram(2)
        ones(2)
        gram(3)
        ones(3)

        # ||p||^2 half A on ACT (idle window), accumulate along d
        sq_dumpA = sb.tile([CL, 512], bf16)
        pnA = sb.tile([CL, 1], f32)
        nc.scalar.activation(sq_dumpA[:], psbA[:], AF.Square, accum_out=pnA[:])

        # ---------------- prototypes: d-quarters 2, 3 -------------------
        for r, pq in ((0, p_q1), (1, p_q2)):
            for jp in range(SJP):
                nc.tensor.matmul(
                    pq[:],
                    oh[:],
                    s8q[:, r, jp],
                    start=(jp == 0),
                    stop=(jp == SJP - 1),
                    perf_mode=DR,
                )
            dummy_mm(1)
            w_quarter(pq[:], psbB, slice(256 * r, 256 * (r + 1)), 4 + 2 * r)
        dummy_mm(1)

        for k in range(4, 8):
            sq(k, 0)
        sq(4, 1)
        sq(5, 1)

        # ||p||^2 half B on DVE (keeps the ACT queue clear for the
        # W-evacs and sqrts in the tail)
        pnsqB = sb.tile([CL, 512], bf16)
        pnB = sb.tile([CL, 1], f32)
        pn = sb.tile([CL, 1], f32)
        nc.vector.tensor_mul(pnsqB[:], psbB[:], psbB[:])
        nc.vector.tensor_reduce(
            pnB[:], pnsqB[:], axis=mybir.AxisListType.X, op=mybir.AluOpType.add
        )
        nc.vector.tensor_add(pn[:], pnA[:], pnB[:])

        sq(6, 1)
        sq(7, 1)

        # tail matmuls for d-half1
        gram(4)
        ones(4)
        gram(5)
        ones(5)
        gram(6)
        ones(6)
        gram(7)
        ones(7, stop=True)

        # ------- sqrt(+||p||^2), negate, store (2 q-halves pipelined) ---
        for qh in range(2):
            qs = slice(512 * qh, 512 * (qh + 1))
            lt = sb.tile([CL, 512], f32, name=f"lt{qh}")
            ltn = sb.tile([CL, 512], bf16, name=f"ltn{qh}")
            nc.scalar.activation(lt[:], s_ps[:, qs], AF.Sqrt, bias=pn[:, 0:1])
            nc.vector.tensor_scalar_mul(ltn[:], lt[:], -1.0)
            nc.sync.dma_start(out[:, qs], ltn[:])


def _build():
    if "nc" in _CACHE:
        return _CACHE["nc"]
    from concourse import bacc, mybir, tile

    f32 = mybir.dt.float32
    bf16 = mybir.dt.bfloat16
    fp8 = mybir.dt.float8e4
    nc = bacc.Bacc(
        "TRN2",
        target_bir_lowering=False,
        debug=False,
        enable_asserts=False,
        num_devices=NCORES,
    )
    sup = nc.dram_tensor("sup", [128, 2 * SJP * 2 * 512], fp8, kind="ExternalInput").ap()
    qt = nc.dram_tensor("qt", [128, DCH * QL], bf16, kind="ExternalInput").ap()
    oh_in = nc.dram_tensor("oh", [128, 2 * CL], fp8, kind="ExternalInput").ap()
    out = nc.dram_tensor("out", [CL, QL], bf16, kind="ExternalOutput").ap()
    with tile.TileContext(nc) as tc:
        _emit(nc, tc, sup, qt, oh_in, out)
    nc.compile()
    _CACHE["nc"] = nc
    return nc


def _shard(embeddings):
    import ml_dtypes

    emb = np.asarray(embeddings, dtype=np.float32).reshape(C, 2 * S, D)
    support = emb[:, :S, :]                       # [64, 64, 1024]
    queries = emb[:, S:, :].reshape(C * S, D)     # [4096, 1024]

    # one-hot: oh[p, o, c] = 1 iff c == p//4 (same for every chunk-pair)
    p = np.arange(128)[:, None, None]
    c = np.arange(CL)[None, None, :]
    oh = (c == p // 4) + np.zeros((1, 2, 1), dtype=bool)
    oh = np.ascontiguousarray(
        oh.astype(ml_dtypes.float8_e4m3).reshape(128, 2 * CL)
    )

    sups = []
    for j in range(CB):
        # [32, 64, 1024] -> [c, jp, o, m, d]; regions: d-half0 then two
        # d-quarters, each [c, m, jp, o, dcols] with p = 4c+m
        sj = support[CL * j : CL * (j + 1)].reshape(CL, SJP, 2, 4, D)
        parts = []
        for d0, d1 in ((0, 512), (512, 768), (768, 1024)):
            pr = sj[..., d0:d1].transpose(0, 3, 1, 2, 4)
            parts.append(pr.reshape(128, SJP * 2 * (d1 - d0)))
        sj = np.concatenate(parts, axis=1)
        sups.append(np.ascontiguousarray(sj.astype(ml_dtypes.float8_e4m3)))
    qts = []
    for i in range(QB):
        # [1024q, 1024d] -> T -> [8k, 128p, 1024q] -> [p, k, q]
        qi = queries[QL * i : QL * (i + 1)].T.reshape(DCH, 128, 2, 512)
        qi = qi.transpose(1, 2, 0, 3).reshape(128, DCH * QL)  # [p, qh, k, q]
        qts.append(np.ascontiguousarray(qi.astype(ml_dtypes.bfloat16)))

    in_maps = []
    for k in range(NCORES):
        i, j = k // CB, k % CB
        in_maps.append({"sup": sups[j], "qt": qts[i], "oh": oh})
    return in_maps


def _gather(outs):
    """outs: list of 8 per-core [CL, QL] arrays -> full [C*S, C] f32."""
    logits = np.empty((C * S, C), dtype=np.float32)
    for k in range(NCORES):
        i, j = k // CB, k % CB
        logits[QL * i : QL * (i + 1), CL * j : CL * (j + 1)] = np.asarray(
            outs[k], dtype=np.float32
        ).T
    return logits


def kernel(embeddings_stacked, n_classes, n_support, **_unused):
    assert int(n_classes) == C and int(n_support) == S
    emb = np.asarray(embeddings_stacked)
    assert emb.shape == (C * 2 * S, D), emb.shape

    from concourse import bass_utils

    nc = _build()
    in_maps = _shard(emb)
    try:
        res = bass_utils.run_bass_kernel_spmd(
            nc, in_maps, core_ids=list(range(NCORES))
        )
    except Exception:
        # transient device/runtime hiccups have been observed; retry once
        res = bass_utils.run_bass_kernel_spmd(
            nc, in_maps, core_ids=list(range(NCORES))
        )
    return _gather([res.results[k]["out"] for k in range(NCORES)])


if __name__ == "__main__":
    rng = np.random.default_rng(0)
    emb = rng.standard_normal((C * 2 * S, D), dtype=np.float32)
    got = kernel(emb, C, S)
    print("kernel output", got.shape, got.dtype)


# revision 23
# speedup vs baseline: 1.1820x; 1.0922x over previous
"""Trainium2 Bass kernel for BaselineProtonet (retrieval_knn).

logits[q, c] = -||query_q - proto_c||_2
  proto_c = mean of 64 support embeddings of class c
  embeddings_stacked: [64 classes * (64 support + 64 query), 1024] f32

Sharding (8 cores): query-sharded, support-replicated. Core i owns query
rows 512i..512(i+1); every core receives the full support set (fp8 on
the wire) and computes all 64 prototypes locally on the TensorEngine, so
no cross-core collective is needed (a ncfw collective costs ~50us of
control latency in this runtime, far more than the extra DMA).

Host-side shard prep (layout/encoding only, no arithmetic): support is
pre-swizzled to the exact SBUF layout (contiguous per-partition runs so
HWDGE descriptor generation is cheap) and encoded fp8e4m3; queries are
transposed to feature-major (d on partitions) and encoded bf16.

Per core:
  protos   : 32 fp8 DoubleRow one-hot matmuls (256 support rows each)
             accumulate class sums -> PSUM [64,1024] f32, scaled 1/64
             on ACT evacuation -> bf16 prototypes
  P^T      : 8 PE transposes -> W = -2*P^T (bf16, ACT scale)
  ||p||^2  : ACT square-accumulate on prototypes -> [64,1] f32, added
             per-partition (class) via the ACT sqrt bias
  ||q||^2  : DVE squares + all-ones-stationary matmuls accumulated
             straight into the Gram PSUM group (broadcasts sum_d q_d^2
             to every class row); these open the group and track the
             query stream while the W chain completes
  Gram     : 8 accumulating matmuls lhsT=W chunk, rhs=Q^T chunk (bf16)
  logits   : -sqrt(dist^2) via ACT sqrt(+bias) and DVE negate, in two
             pipelined query halves; output [64, 512] (class-major);
             the host transposes/concats the per-core blocks.
PE is pre-warmed with dummy matmuls during the DMA wait (HAM clock gate)
and the sqrt ACT table is preloaded by a dummy activation.
"""

import numpy as np

C = 64          # classes
S = 64          # support per class (== queries per class)
D = 1024        # embedding dim
NCORES = 8
CL = C // NCORES            # 8 classes per core's query shard
QL = CL * S                 # 512 query rows per core
DCH = D // 128              # 8 d-chunks
SCH = (C * S) // 128        # 32 support row chunks (full support)

_CACHE = {}


def _emit(nc, tc, sup, qt, oh_in, out):
    """Emit the per-core tile program.

    sup:   [128, SCH*D] fp8 DRAM  (full support, swizzled: row p of
                                   chunk j holds support row j*128+p)
    qt:    [128, DCH*QL] bf16 DRAM (queries, swizzled feature-major)
    oh_in: [128, SCH*C] fp8 DRAM  (DoubleRow one-hot class masks)
    out:   [C, QL] f32 DRAM       (negated distances, class-major)
    """
    from concourse import masks, mybir

    f32 = mybir.dt.float32
    bf16 = mybir.dt.bfloat16
    fp8 = mybir.dt.float8e4
    AF = mybir.ActivationFunctionType

    with (
        tc.tile_pool(name="sb", bufs=1) as sb,
        tc.tile_pool(name="ps", bufs=1, space="PSUM") as ps,
    ):
        # warm the PE clock first-thing (HAM gate needs ~3.5us of busy
        # before the real matmuls; deps are a single DVE memset)
        wm_in = sb.tile([128, 512], bf16)
        nc.vector.memset(wm_in[:], 0.0)
        wm_ps = ps.tile([128, 512], f32)
        for _ in range(7):
            nc.tensor.matmul(
                wm_ps[:], wm_in[:, 0:128], wm_in[:], start=True, stop=True
            )


        # ---------------- input DMAs (one sync-ring FIFO: one-hot, the
        # support stream, then query quarters -- slices drain in order;
        # the prototype matmuls track the support stream and the
        # Gram/norm matmuls track the query stream)
        oh = sb.tile([128, 2, C], fp8)
        nc.scalar.dma_start(
            oh[:], oh_in[:, :].rearrange("p (o k) -> p o k", o=2)
        )
        s8 = sb.tile([128, SCH, D], fp8)
        for b in range(8):
            nc.sync.dma_start(
                s8[:, 4 * b : 4 * (b + 1)],
                sup[:, 4 * b * D : 4 * (b + 1) * D].rearrange(
                    "p (c d) -> p c d", c=4
                ),
            )
        q16 = sb.tile([128, DCH, QL], bf16)
        for h in range(2):
            nc.sync.dma_start(
                q16[:, 4 * h : 4 * (h + 1)],
                qt[:, 4 * h * QL : 4 * (h + 1) * QL].rearrange(
                    "p (k q) -> p k q", k=4
                ),
            )

        # ---------------- constants -------------------------------------
        ident = sb.tile([128, 128], bf16)
        masks.make_identity(nc, ident[:])
        ones64 = sb.tile([128, C], bf16)
        nc.gpsimd.memset(ones64[:], 1.0)

        # preload the sqrt ACT table set off the critical path
        warm_sq = sb.tile([1, 1], f32)
        nc.gpsimd.memset(warm_sq[:], 1.0)
        nc.scalar.activation(warm_sq[:], warm_sq[:], AF.Sqrt)

        # ---------------- prototypes (all 64 classes) -------------------
        # fp8 DoubleRow: each matmul contracts 256 support rows (chunk
        # pair jp), streaming 2 rows/cycle through the PE array
        s8v = s8[:].rearrange("p (jp o) d -> p jp o d", o=2)
        p_ps = ps.tile([C, D], f32)  # [64, 1024] = 2 banks
        for jp in range(SCH // 2):
            for h in range(2):
                nc.tensor.matmul(
                    p_ps[:, 512 * h : 512 * (h + 1)],
                    oh[:],
                    s8v[:, jp, :, 512 * h : 512 * (h + 1)],
                    start=(jp == 0),
                    stop=(jp == SCH // 2 - 1),
                    perf_mode=mybir.MatmulPerfMode.DoubleRow,
                )
        # ---------------- ||q||^2 squares (per chunk, DVE) ---------------
        qsq = sb.tile([128, DCH, QL], bf16)
        for k in range(DCH):
            nc.vector.tensor_mul(qsq[:, k], q16[:, k], q16[:, k])

        # ||q||^2 matmuls open the s_ps PSUM group and track the query
        # stream while the W chain (evac -> transpose -> scale) completes
        # on ACT/PE; the Gram matmuls are appended after W below.
        # s_ps[c, q] = sum_k ( ones^T qsq_k + W_k^T q_k ) = ||q||^2 - 2 q.p
        s_ps = ps.tile([C, QL], f32)
        for k in range(DCH):
            nc.tensor.matmul(
                s_ps[:], ones64[:], qsq[:, k], start=(k == 0), stop=False
            )

        # evacuate the two d-halves on ACT (DVE is busy with the query
        # squares; separate tiles so Tile doesn't serialize the writers)
        psbA = sb.tile([C, 512], bf16)
        psbB = sb.tile([C, 512], bf16)
        nc.scalar.mul(psbA[:], p_ps[:, 0:512], 1.0 / S)
        nc.scalar.mul(psbB[:], p_ps[:, 512:1024], 1.0 / S)

        # ---------------- W = -2 * P^T (bf16, ACT evac) ------------------
        pt_ps = ps.tile([128, DCH * C], bf16)  # chunk k at cols 64k..64k+64
        W = sb.tile([128, DCH, C], bf16)
        # W evac split per d-half so the first gram matmuls start after
        # only 4 transposes + half the ACT evac, not the full chain
        for hk in range(2):
            half = psbA if hk == 0 else psbB
            for kk in range(4):
                k = 4 * hk + kk
                nc.tensor.transpose(
                    pt_ps[:, C * k : C * (k + 1)],
                    half[:, 128 * kk : 128 * (kk + 1)],
                    ident[0:C, 0:C],
                )
            nc.scalar.mul(
                W[:, 4 * hk : 4 * (hk + 1)].rearrange("p k c -> p (k c)"),
                pt_ps[:, 4 * C * hk : 4 * C * (hk + 1)],
                -2.0,
            )

        # ||p||^2 in f32 via ACT square-accumulate (consistent with the
        # bf16 protos used in the Gram); halves summed on DVE
        pn_dump = sb.tile([C, D], bf16)
        pnA = sb.tile([C, 1], f32)
        pnB = sb.tile([C, 1], f32)
        pn_col = sb.tile([C, 1], f32)
        nc.scalar.activation(pn_dump[:, 0:512], psbA[:], AF.Square, accum_out=pnA[:])
        nc.scalar.activation(
            pn_dump[:, 512:1024], psbB[:], AF.Square, accum_out=pnB[:]
        )

        # ------- Gram matmuls (follow the W chain) -----------------------
        for k in range(DCH):
            nc.tensor.matmul(
                s_ps[:], W[:, k], q16[:, k], start=False, stop=(k == DCH - 1)
            )

        nc.vector.tensor_add(pn_col[:], pnA[:], pnB[:])

        # ------- sqrt(+||p||^2), negate, store (2 q-halves pipelined) ----
        lt = sb.tile([C, QL], f32)
        ltn = sb.tile([C, QL], bf16)
        for hq in range(2):
            s = slice(256 * hq, 256 * (hq + 1))
            nc.scalar.activation(lt[:, s], s_ps[:, s], AF.Sqrt, bias=pn_col[:, 0:1])
            nc.vector.tensor_scalar_mul(ltn[:, s], lt[:, s], -1.0)
            nc.sync.dma_start(out[:, s], ltn[:, s])


def _build():
    if "nc" in _CACHE:
        return _CACHE["nc"]
    from concourse import bacc, mybir, tile

    f32 = mybir.dt.float32
    bf16 = mybir.dt.bfloat16
    fp8 = mybir.dt.float8e4
    nc = bacc.Bacc(
        "TRN2",
        target_bir_lowering=False,
        debug=False,
        enable_asserts=False,
        num_devices=NCORES,
    )
    sup = nc.dram_tensor("sup", [128, SCH * D], fp8, kind="ExternalInput").ap()
    qt = nc.dram_tensor("qt", [128, DCH * QL], bf16, kind="ExternalInput").ap()
    oh_in = nc.dram_tensor("oh", [128, 2 * C], fp8, kind="ExternalInput").ap()
    out = nc.dram_tensor("out", [C, QL], bf16, kind="ExternalOutput").ap()
    with tile.TileContext(nc) as tc:
        _emit(nc, tc, sup, qt, oh_in, out)
    nc.compile()
    _CACHE["nc"] = nc
    return nc


def _onehot():
    import ml_dtypes

    # DoubleRow one-hot, identical for every chunk-pair: class (2p+o)//4
    # owns slot (p, o); 4 slots per class per chunk-pair * 16 pairs = 64
    p = np.arange(128)[:, None, None]
    o = np.arange(2)[None, :, None]
    c = np.arange(C)[None, None, :]
    oh = (c == (2 * p + o) // 4).astype(ml_dtypes.float8_e4m3)
    return np.ascontiguousarray(oh.reshape(128, 2 * C))


def _shard(embeddings):
    import ml_dtypes

    emb = np.asarray(embeddings, dtype=np.float32).reshape(C, 2 * S, D)
    # support swizzle matching the interleaved one-hot: slot (p, j) of
    # the SBUF tile holds support row (class=(2p+j%2)//4,
    # idx=(j//2)*4 + (2p+j%2)%4), fp8 on the wire
    sup_cs = emb[:, :S, :]                      # [C, S, D]
    p = np.arange(128)[:, None]
    j = np.arange(SCH)[None, :]
    t = 2 * p + (j % 2)
    sup = sup_cs[t // 4, (j // 2) * 4 + (t % 4)]   # [128, SCH, D]
    sup = np.ascontiguousarray(
        sup.astype(ml_dtypes.float8_e4m3).reshape(128, SCH * D)
    )
    oh = _onehot()
    in_maps = []
    for i in range(NCORES):
        q = emb[CL * i : CL * (i + 1), S:, :].reshape(QL, D)
        # Q^T [D, QL] -> swizzled [128, DCH, QL] bf16
        qt_i = q.T.reshape(DCH, 128, QL).transpose(1, 0, 2)
        qt_i = np.ascontiguousarray(
            qt_i.astype(ml_dtypes.bfloat16).reshape(128, DCH * QL)
        )
        in_maps.append({"sup": sup, "qt": qt_i, "oh": oh})
    return in_maps


def _gather(outs):
    """outs: list of 8 per-core [C, QL] arrays -> full [C*S, C] f32."""
    logits = np.empty((C * S, C), dtype=np.float32)
    for i in range(NCORES):
        logits[QL * i : QL * (i + 1), :] = np.asarray(
            outs[i], dtype=np.float32
        ).T
    return logits


def kernel(embeddings_stacked, n_classes, n_support, **_unused):
    assert int(n_classes) == C and int(n_support) == S
    emb = np.asarray(embeddings_stacked)
    assert emb.shape == (C * 2 * S, D), emb.shape

    from concourse import bass_utils

    nc = _build()
    in_maps = _shard(emb)
    try:
        res = bass_utils.run_bass_kernel_spmd(
            nc, in_maps, core_ids=list(range(NCORES))
        )
    except Exception:
        # transient device/runtime hiccups have been observed; retry once
        res = bass_utils.run_bass_kernel_spmd(
            nc, in_maps, core_ids=list(range(NCORES))
        )
    return _gather([res.results[i]["out"] for i in range(NCORES)])


if __name__ == "__main__":
    rng = np.random.default_rng(0)
    emb = rng.standard_normal((C * 2 * S, D), dtype=np.float32)
    got = kernel(emb, C, S)
    print("kernel output", got.shape, got.dtype)



# revision 25
# speedup vs baseline: 1.1843x; 1.0019x over previous
"""Trainium2 Bass kernel for BaselineProtonet (retrieval_knn).

logits[q, c] = -||query_q - proto_c||_2
  proto_c = mean of 64 support embeddings of class c
  embeddings_stacked: [64 classes * (64 support + 64 query), 1024] f32

Sharding (8 cores): query-sharded, support-replicated. Core i owns query
rows 512i..512(i+1); every core receives the full support set (fp8 on
the wire) and computes all 64 prototypes locally on the TensorEngine, so
no cross-core collective is needed (a ncfw collective costs ~50us of
control latency in this runtime, far more than the extra DMA).

Host-side shard prep (layout/encoding only, no arithmetic): support is
pre-swizzled to the exact SBUF layout (contiguous per-partition runs so
HWDGE descriptor generation is cheap) and encoded fp8e4m3; queries are
transposed to feature-major (d on partitions) and encoded bf16.

Per core:
  protos   : 32 fp8 DoubleRow one-hot matmuls (256 support rows each)
             accumulate class sums -> PSUM [64,1024] f32, scaled 1/64
             on ACT evacuation -> bf16 prototypes
  P^T      : 8 PE transposes -> W = -2*P^T (bf16, ACT scale)
  ||p||^2  : ACT square-accumulate on prototypes -> [64,1] f32, added
             per-partition (class) via the ACT sqrt bias
  ||q||^2  : DVE squares + all-ones-stationary matmuls accumulated
             straight into the Gram PSUM group (broadcasts sum_d q_d^2
             to every class row); these open the group and track the
             query stream while the W chain completes
  Gram     : 8 accumulating matmuls lhsT=W chunk, rhs=Q^T chunk (bf16)
  logits   : -sqrt(dist^2) via ACT sqrt(+bias) and DVE negate, in two
             pipelined query halves; output [64, 512] (class-major);
             the host transposes/concats the per-core blocks.
PE is pre-warmed with dummy matmuls during the DMA wait (HAM clock gate)
and the sqrt ACT table is preloaded by a dummy activation.
"""

import numpy as np

C = 64          # classes
S = 64          # support per class (== queries per class)
D = 1024        # embedding dim
NCORES = 8
CL = C // NCORES            # 8 classes per core's query shard
QL = CL * S                 # 512 query rows per core
DCH = D // 128              # 8 d-chunks
SCH = (C * S) // 128        # 32 support row chunks (full support)

_CACHE = {}


def _emit(nc, tc, sup, qt, oh_in, out):
    """Emit the per-core tile program.

    sup:   [128, SCH*D] fp8 DRAM  (full support, swizzled: row p of
                                   chunk j holds support row j*128+p)
    qt:    [128, DCH*QL] bf16 DRAM (queries, swizzled feature-major)
    oh_in: [128, SCH*C] fp8 DRAM  (DoubleRow one-hot class masks)
    out:   [C, QL] f32 DRAM       (negated distances, class-major)
    """
    from concourse import masks, mybir

    f32 = mybir.dt.float32
    bf16 = mybir.dt.bfloat16
    fp8 = mybir.dt.float8e4
    AF = mybir.ActivationFunctionType

    with (
        tc.tile_pool(name="sb", bufs=1) as sb,
        tc.tile_pool(name="ps", bufs=1, space="PSUM") as ps,
    ):
        # warm the PE clock first-thing (HAM gate needs ~3.5us of busy
        # before the real matmuls; deps are a single DVE memset)
        wm_in = sb.tile([128, 512], bf16)
        nc.vector.memset(wm_in[:], 0.0)
        wm_ps = ps.tile([128, 512], f32)
        for _ in range(7):
            nc.tensor.matmul(
                wm_ps[:], wm_in[:, 0:128], wm_in[:], start=True, stop=True
            )


        # ---------------- input DMAs (one sync-ring FIFO: one-hot, the
        # support stream, then query quarters -- slices drain in order;
        # the prototype matmuls track the support stream and the
        # Gram/norm matmuls track the query stream)
        oh = sb.tile([128, 2, C], fp8)
        nc.scalar.dma_start(
            oh[:], oh_in[:, :].rearrange("p (o k) -> p o k", o=2)
        )
        s8 = sb.tile([128, SCH, D], fp8)
        for b in range(8):
            nc.sync.dma_start(
                s8[:, 4 * b : 4 * (b + 1)],
                sup[:, 4 * b * D : 4 * (b + 1) * D].rearrange(
                    "p (c d) -> p c d", c=4
                ),
            )
        q16 = sb.tile([128, DCH, QL], bf16)
        for h in range(2):
            nc.sync.dma_start(
                q16[:, 4 * h : 4 * (h + 1)],
                qt[:, 4 * h * QL : 4 * (h + 1) * QL].rearrange(
                    "p (k q) -> p k q", k=4
                ),
            )

        # ---------------- constants -------------------------------------
        ident = sb.tile([128, 128], bf16)
        masks.make_identity(nc, ident[:])
        ones64 = sb.tile([128, C], bf16)
        nc.gpsimd.memset(ones64[:], 1.0)

        # preload the sqrt ACT table set off the critical path
        warm_sq = sb.tile([1, 1], f32)
        nc.gpsimd.memset(warm_sq[:], 1.0)
        nc.scalar.activation(warm_sq[:], warm_sq[:], AF.Sqrt)

        # ---------------- prototypes (all 64 classes) -------------------
        # fp8 DoubleRow: each matmul contracts 256 support rows (chunk
        # pair jp), streaming 2 rows/cycle through the PE array
        s8v = s8[:].rearrange("p (jp o) d -> p jp o d", o=2)
        p_ps = ps.tile([C, D], f32)  # [64, 1024] = 2 banks
        for jp in range(SCH // 2):
            for h in range(2):
                nc.tensor.matmul(
                    p_ps[:, 512 * h : 512 * (h + 1)],
                    oh[:],
                    s8v[:, jp, :, 512 * h : 512 * (h + 1)],
                    start=(jp == 0),
                    stop=(jp == SCH // 2 - 1),
                    perf_mode=mybir.MatmulPerfMode.DoubleRow,
                )
        # ---------------- ||q||^2 squares (per chunk, DVE) ---------------
        qsq = sb.tile([128, DCH, QL], bf16)
        for k in range(DCH):
            nc.vector.tensor_mul(qsq[:, k], q16[:, k], q16[:, k])

        # ||q||^2 matmuls open the s_ps PSUM group and track the query
        # stream while the W chain (evac -> transpose -> scale) completes
        # on ACT/PE; the Gram matmuls are appended after W below.
        # s_ps[c, q] = sum_k ( ones^T qsq_k + W_k^T q_k ) = ||q||^2 - 2 q.p
        s_ps = ps.tile([C, QL], f32)
        for k in range(DCH):
            nc.tensor.matmul(
                s_ps[:], ones64[:], qsq[:, k], start=(k == 0), stop=False
            )

        # evacuate the two d-halves on ACT (DVE is busy with the query
        # squares; separate tiles so Tile doesn't serialize the writers)
        psbA = sb.tile([C, 512], bf16)
        psbB = sb.tile([C, 512], bf16)
        nc.scalar.mul(psbA[:], p_ps[:, 0:512], 1.0 / S)
        nc.scalar.mul(psbB[:], p_ps[:, 512:1024], 1.0 / S)

        # ---------------- W = -2 * P^T (bf16, ACT evac) ------------------
        pt_ps = ps.tile([128, DCH * C], bf16)  # chunk k at cols 64k..64k+64
        W = sb.tile([128, DCH, C], bf16)
        # W evac split per d-half so the first gram matmuls start after
        # only 4 transposes + half the ACT evac, not the full chain
        for hk in range(2):
            half = psbA if hk == 0 else psbB
            for kk in range(4):
                k = 4 * hk + kk
                nc.tensor.transpose(
                    pt_ps[:, C * k : C * (k + 1)],
                    half[:, 128 * kk : 128 * (kk + 1)],
                    ident[0:C, 0:C],
                )
            nc.scalar.mul(
                W[:, 4 * hk : 4 * (hk + 1)].rearrange("p k c -> p (k c)"),
                pt_ps[:, 4 * C * hk : 4 * C * (hk + 1)],
                -2.0,
            )

        # ||p||^2 in f32 via ACT square-accumulate (consistent with the
        # bf16 protos used in the Gram); halves summed on DVE
        pn_dump = sb.tile([C, D], bf16)
        pnA = sb.tile([C, 1], f32)
        pnB = sb.tile([C, 1], f32)
        pn_col = sb.tile([C, 1], f32)
        nc.scalar.activation(pn_dump[:, 0:512], psbA[:], AF.Square, accum_out=pnA[:])
        nc.scalar.activation(
            pn_dump[:, 512:1024], psbB[:], AF.Square, accum_out=pnB[:]
        )

        # ------- Gram matmuls (follow the W chain) -----------------------
        for k in range(DCH):
            nc.tensor.matmul(
                s_ps[:], W[:, k], q16[:, k], start=False, stop=(k == DCH - 1)
            )

        nc.vector.tensor_add(pn_col[:], pnA[:], pnB[:])

        # ------- sqrt(+||p||^2), negate, store (2 q-halves pipelined) ----
        lt = sb.tile([C, QL], f32)
        ltn = sb.tile([C, QL], bf16)
        for hq in range(2):
            s = slice(256 * hq, 256 * (hq + 1))
            nc.scalar.activation(lt[:, s], s_ps[:, s], AF.Sqrt, bias=pn_col[:, 0:1])
            nc.vector.tensor_scalar_mul(ltn[:, s], lt[:, s], -1.0)
            nc.sync.dma_start(out[:, s], ltn[:, s])


def _build():
    if "nc" in _CACHE:
        return _CACHE["nc"]
    from concourse import bacc, mybir, tile

    f32 = mybir.dt.float32
    bf16 = mybir.dt.bfloat16
    fp8 = mybir.dt.float8e4
    nc = bacc.Bacc(
        "TRN2",
        target_bir_lowering=False,
        debug=False,
        enable_asserts=False,
        num_devices=NCORES,
    )
    sup = nc.dram_tensor("sup", [128, SCH * D], fp8, kind="ExternalInput").ap()
    qt = nc.dram_tensor("qt", [128, DCH * QL], bf16, kind="ExternalInput").ap()
    oh_in = nc.dram_tensor("oh", [128, 2 * C], fp8, kind="ExternalInput").ap()
    out = nc.dram_tensor("out", [C, QL], bf16, kind="ExternalOutput").ap()
    with tile.TileContext(nc) as tc:
        _emit(nc, tc, sup, qt, oh_in, out)
    nc.compile()
    _CACHE["nc"] = nc
    return nc


def _onehot():
    import ml_dtypes

    # DoubleRow one-hot, identical for every chunk-pair: class (2p+o)//4
    # owns slot (p, o); 4 slots per class per chunk-pair * 16 pairs = 64
    p = np.arange(128)[:, None, None]
    o = np.arange(2)[None, :, None]
    c = np.arange(C)[None, None, :]
    oh = (c == (2 * p + o) // 4).astype(ml_dtypes.float8_e4m3)
    return np.ascontiguousarray(oh.reshape(128, 2 * C))


def _shard(embeddings):
    import ml_dtypes

    emb = np.asarray(embeddings, dtype=np.float32).reshape(C, 2 * S, D)
    # support swizzle matching the interleaved one-hot: slot (p, j) of
    # the SBUF tile holds support row (class=(2p+j%2)//4,
    # idx=(j//2)*4 + (2p+j%2)%4), fp8 on the wire
    sup_cs = emb[:, :S, :]                      # [C, S, D]
    p = np.arange(128)[:, None]
    j = np.arange(SCH)[None, :]
    t = 2 * p + (j % 2)
    sup = sup_cs[t // 4, (j // 2) * 4 + (t % 4)]   # [128, SCH, D]
    sup = np.ascontiguousarray(
        sup.astype(ml_dtypes.float8_e4m3).reshape(128, SCH * D)
    )
    oh = _onehot()
    in_maps = []
    for i in range(NCORES):
        q = emb[CL * i : CL * (i + 1), S:, :].reshape(QL, D)
        # Q^T [D, QL] -> swizzled [128, DCH, QL] bf16
        qt_i = q.T.reshape(DCH, 128, QL).transpose(1, 0, 2)
        qt_i = np.ascontiguousarray(
            qt_i.astype(ml_dtypes.bfloat16).reshape(128, DCH * QL)
        )
        in_maps.append({"sup": sup, "qt": qt_i, "oh": oh})
    return in_maps


def _gather(outs):
    """outs: list of 8 per-core [C, QL] arrays -> full [C*S, C] f32."""
    logits = np.empty((C * S, C), dtype=np.float32)
    for i in range(NCORES):
        logits[QL * i : QL * (i + 1), :] = np.asarray(
            outs[i], dtype=np.float32
        ).T
    return logits


def kernel(embeddings_stacked, n_classes, n_support, **_unused):
    assert int(n_classes) == C and int(n_support) == S
    emb = np.asarray(embeddings_stacked)
    assert emb.shape == (C * 2 * S, D), emb.shape

    from concourse import bass_utils

    nc = _build()
    in_maps = _shard(emb)
    try:
        res = bass_utils.run_bass_kernel_spmd(
            nc, in_maps, core_ids=list(range(NCORES))
        )
    except Exception:
        # transient device/runtime hiccups have been observed; retry once
        res = bass_utils.run_bass_kernel_spmd(
            nc, in_maps, core_ids=list(range(NCORES))
        )
    return _gather([res.results[i]["out"] for i in range(NCORES)])


if __name__ == "__main__":
    rng = np.random.default_rng(0)
    emb = rng.standard_normal((C * 2 * S, D), dtype=np.float32)
    got = kernel(emb, C, S)
    print("kernel output", got.shape, got.dtype)

